# revision 7
# baseline (speedup 1.0000x reference)
"""Trainium2 Bass kernel for a 2-hop GCN-normalized aggregation (Nettack surrogate).

Computes h = A_hat^2 (x @ W) with A_hat = D^-1/2 (A + I) D^-1/2, where A is the
(self-loop-stripped) adjacency from edge_index, on 8 NeuronCores.

Strategy:
  - Nodes are partitioned row-wise across 8 cores (12500 rows each).
  - u = dinv * (x @ W) computed per-core (x slice shipped pre-transposed, fp16),
    stored as an fp16 table padded to 128 channels (256B rows), AllGather ->
    full u table per core.
  - Each hop: per destination-node block of 128, gather u[col] rows for that
    block's edges via SWDGE dma_gather (int16 indices => 4 source sub-ranges,
    spread over the 4 SWDGE queue pairs), build a 0/1 one-hot (edge -> local
    row) with DVE is_equal in fp16, segment-sum via fp16 PE matmul accumulating
    in fp32 PSUM. Add self term, scale by dinv^2 (hop1) / dinv (hop2).
  - AllGather between hops; final f32 result is the output slice per core.

Host-side preprocessing is limited to edge metadata: bucketing/padding edge
indices, degree counts, and layout packing. All FLOPs on x/W run on device.
"""

import numpy as np


# ---------------------------------------------------------------- parameters

def full_params():
    return dict(
        N=100000, E=3200000, IN_CH=512, CH=64, NCORES=8,
        SBB=7,    # 128-row blocks per superblock (gather granularity)
        NR=4,     # int16 index sub-ranges
    )


def derive_params(p):
    q = dict(p)
    q["RPC"] = p["N"] // p["NCORES"]                      # rows per core
    q["NBLK"] = -(-q["RPC"] // 128)                       # 128-row blocks per core
    assert q["NBLK"] % p["SBB"] == 0, (q["NBLK"], p["SBB"])
    q["NSB"] = q["NBLK"] // p["SBB"]
    assert p["N"] % p["NR"] == 0
    q["RANGE"] = p["N"] // p["NR"]
    assert q["RANGE"] <= 32768
    q["KT"] = p["IN_CH"] // 128
    assert p["IN_CH"] % 128 == 0
    q["TW"] = 128                                         # fp16 table row width (256B)
    assert p["CH"] <= q["TW"]
    return q


# ---------------------------------------------------------------- host prep

def preprocess(edge_index, p):
    """Edge metadata -> per-core packed index/one-hot-row/scale arrays."""
    N, NCORES, NR = p["N"], p["NCORES"], p["NR"]
    RPC, NBLK, SBB, NSB, RANGE = p["RPC"], p["NBLK"], p["SBB"], p["NSB"], p["RANGE"]

    row = np.asarray(edge_index[0], dtype=np.int64)
    col = np.asarray(edge_index[1], dtype=np.int64)
    keep = row != col
    row, col = row[keep], col[keep]

    deg = np.bincount(row, minlength=N).astype(np.float64) + 1.0
    dinv = (1.0 / np.sqrt(deg)).astype(np.float32)

    core = row // RPC
    lrow = row - core * RPC
    blk = lrow >> 7
    rloc = (lrow & 127).astype(np.float32)
    rng_ = col // RANGE

    bucket = (core * NBLK + blk) * NR + rng_
    counts = np.bincount(bucket, minlength=NCORES * NBLK * NR)
    CPB = max(1, int(-(-counts.max() // 128)))
    CAP = CPB * 128

    order = np.argsort(bucket, kind="stable")
    sortb = bucket[order]
    starts = np.zeros(len(counts) + 1, np.int64)
    np.cumsum(counts, out=starts[1:])
    within = np.arange(len(row), dtype=np.int64) - starts[sortb]

    c_s = sortb // (NBLK * NR)
    rem = sortb % (NBLK * NR)
    blk_s = rem // NR
    r_s = rem % NR
    sb_s = blk_s // SBB
    bin_s = blk_s % SBB
    F = ((sb_s * NR + r_s) * SBB + bin_s) * CAP + within

    L = NSB * NR * SBB * CAP
    gidx = np.zeros((NCORES, L), np.int16)            # pad: valid dummy row 0
    rl = np.full((NCORES, L), -1.0, np.float32)       # pad: matches no row
    gidx[c_s, F] = (col[order] - r_s * RANGE).astype(np.int16)
    rl[c_s, F] = rloc[order]

    # wrap: idx i -> [i%16, i//16], replicated into all 8 groups of 16 rows
    gidx_w = np.zeros((NCORES, 128, L // 16), np.int16)
    base = gidx.reshape(NCORES, L // 16, 16).transpose(0, 2, 1)
    for g in range(8):
        gidx_w[:, g * 16:(g + 1) * 16, :] = base
    rloc_w = np.ascontiguousarray(rl.reshape(NCORES, L // 128, 128).transpose(0, 2, 1))

    # per-core dinv / dinv^2 in [128, NBLK] block layout (pad rows -> 0)
    dpad = np.zeros(NCORES * NBLK * 128, np.float32)
    dpad_view = dpad.reshape(NCORES, NBLK * 128)
    for c in range(NCORES):
        dpad_view[c, :RPC] = dinv[c * RPC:(c + 1) * RPC]
    dinv_a = np.ascontiguousarray(dpad.reshape(NCORES, NBLK, 128).transpose(0, 2, 1))
    dinv2_a = dinv_a * dinv_a

    iota = np.broadcast_to(np.arange(128, dtype=np.float16), (128, 128)).copy()
    return dict(CPB=CPB, gidx_w=gidx_w, rloc_w=rloc_w,
                dinv_a=dinv_a, dinv2_a=dinv2_a, iota=iota)


# ---------------------------------------------------------------- device program

def build_nc(p, CPB):
    import concourse.bacc as bacc
    import concourse.mybir as mybir
    from concourse.tile import TileContext

    f32 = mybir.dt.float32
    f16 = mybir.dt.float16
    N, CH, IN_CH, NCORES = p["N"], p["CH"], p["IN_CH"], p["NCORES"]
    RPC, NBLK, SBB, NSB, NR, RANGE, KT, TW = (
        p["RPC"], p["NBLK"], p["SBB"], p["NSB"], p["NR"], p["RANGE"], p["KT"],
        p["TW"])
    CAP = CPB * 128
    SECT = SBB * CPB            # chunk columns per (sb, range) section
    L = NSB * NR * SBB * CAP    # per-core edge-slot count

    nc = bacc.Bacc("TRN2", num_swdge_queues=4)
    xT_d = nc.declare_dram_parameter("xT", [IN_CH, RPC], f16, isOutput=False)
    W_d = nc.declare_dram_parameter("W", [IN_CH, CH], f16, isOutput=False)
    gidx_d = nc.declare_dram_parameter("gidx", [128, L // 16], mybir.dt.int16, isOutput=False)
    rloc_d = nc.declare_dram_parameter("rloc", [128, L // 128], f32, isOutput=False)
    dinv_d = nc.declare_dram_parameter("dinv_a", [128, NBLK], f32, isOutput=False)
    dinv2_d = nc.declare_dram_parameter("dinv2_a", [128, NBLK], f32, isOutput=False)
    iota_d = nc.declare_dram_parameter("iota", [128, 128], f16, isOutput=False)
    out_d = nc.declare_dram_parameter("out", [RPC, CH], f32, isOutput=True)

    u_self = nc.dram_tensor("u_self", [RPC, TW], f16)
    u2_self = nc.dram_tensor("u2_self", [RPC, TW], f16)
    u_full = nc.dram_tensor("u_full", [N, TW], f16, addr_space="Shared")
    u2_full = nc.dram_tensor("u2_full", [N, TW], f16, addr_space="Shared")

    groups = [list(range(NCORES))]

    with TileContext(nc) as tc:
        with tc.tile_pool(name="const", bufs=1) as cpool:
            iota_sb = cpool.tile([128, 128], f16)
            nc.sync.dma_start(out=iota_sb[:], in_=iota_d[:])
            dinv_sb = cpool.tile([128, NBLK], f32)
            nc.sync.dma_start(out=dinv_sb[:], in_=dinv_d[:])
            dinv2_sb = cpool.tile([128, NBLK], f32)
            nc.sync.dma_start(out=dinv2_sb[:], in_=dinv2_d[:])
            w_sb = cpool.tile([128, KT, CH], f16)
            for k in range(KT):
                nc.sync.dma_start(out=w_sb[:, k, :], in_=W_d[k * 128:(k + 1) * 128, :])

            # ---------------- phase 1: u = dinv * (x @ W) ----------------
            with tc.tile_pool(name="p1", bufs=4) as p1, \
                 tc.tile_pool(name="p1ps", bufs=2, space="PSUM") as p1ps, \
                 tc.tile_pool(name="p1o", bufs=3) as p1o:
                for blk in range(NBLK):
                    nb = min(128, RPC - blk * 128)
                    ps = p1ps.tile([128, CH], f32, tag="ps")
                    for k in range(KT):
                        xt = p1.tile([128, 128], f16, tag="xt")
                        nc.sync.dma_start(
                            out=xt[:, :nb],
                            in_=xT_d[k * 128:(k + 1) * 128, blk * 128:blk * 128 + nb])
                        nc.tensor.matmul(
                            out=ps[:nb, :], lhsT=xt[:, :nb], rhs=w_sb[:, k, :],
                            start=(k == 0), stop=(k == KT - 1))
                    ut = p1o.tile([128, TW], f16, tag="ut")
                    nc.vector.memset(ut[:, CH:], 0.0)
                    nc.vector.tensor_scalar(
                        out=ut[:nb, :CH], in0=ps[:nb, :],
                        scalar1=dinv_sb[:nb, blk:blk + 1], scalar2=None,
                        op0=mybir.AluOpType.mult)
                    nc.sync.dma_start(out=u_self[blk * 128:blk * 128 + nb, :],
                                      in_=ut[:nb, :])

            nc.gpsimd.collective_compute(
                "AllGather", mybir.AluOpType.bypass,
                ins=[u_self[:]], outs=[u_full[:]], replica_groups=groups)

            # ---------------- hops ----------------
            def hop(table, self_tab, scale_sb, dest, dest_f16):
                with tc.tile_pool(name="hidx", bufs=2) as hidx, \
                     tc.tile_pool(name="hrl", bufs=2) as hrl, \
                     tc.tile_pool(name="hg", bufs=2) as hg, \
                     tc.tile_pool(name="hoh", bufs=8) as hoh, \
                     tc.tile_pool(name="hps", bufs=1, space="PSUM") as hps, \
                     tc.tile_pool(name="hfin", bufs=3) as hfin:
                    for sb in range(NSB):
                        cpsb = NR * SECT          # chunk cols per sb
                        idx_sb = hidx.tile([128, cpsb * 8], mybir.dt.int16, tag="idx")
                        nc.sync.dma_start(
                            out=idx_sb[:],
                            in_=gidx_d[:, sb * cpsb * 8:(sb + 1) * cpsb * 8])
                        rl_sb = hrl.tile([128, cpsb], f32, tag="rl")
                        nc.sync.dma_start(
                            out=rl_sb[:],
                            in_=rloc_d[:, sb * cpsb:(sb + 1) * cpsb])
                        psums = [hps.tile([128, CH], f32, tag=f"hp{b}",
                                          name=f"hp{b}")
                                 for b in range(SBB)]
                        for r in range(NR):
                            g = hg.tile([128, SECT, TW], f16, tag=f"g{r}",
                                        name=f"g{r}")
                            nc.gpsimd.dma_gather(
                                g[:], table[r * RANGE:(r + 1) * RANGE, :],
                                idx_sb[:, r * SECT * 8:(r + 1) * SECT * 8],
                                num_idxs=SECT * 128, num_idxs_reg=SECT * 128,
                                elem_size=TW, single_packet=False,
                                queue_num=r)
                            for b in range(SBB):
                                for j in range(CPB):
                                    ch = b * CPB + j
                                    oh = hoh.tile([128, 128], f16, tag="oh")
                                    nc.vector.tensor_scalar(
                                        out=oh[:], in0=iota_sb[:],
                                        scalar1=rl_sb[:, r * SECT + ch:
                                                      r * SECT + ch + 1],
                                        scalar2=None,
                                        op0=mybir.AluOpType.is_equal)
                                    nc.tensor.matmul(
                                        out=psums[b][:, :],
                                        lhsT=oh[:],
                                        rhs=g[:, ch, :CH],
                                        start=(r == 0 and j == 0),
                                        stop=(r == NR - 1 and j == CPB - 1))
                        for b in range(SBB):
                            blk = sb * SBB + b
                            nb = min(128, RPC - blk * 128)
                            if nb <= 0:
                                continue
                            st = hfin.tile([128, CH], f16, tag="st")
                            nc.sync.dma_start(
                                out=st[:nb, :],
                                in_=self_tab[blk * 128:blk * 128 + nb, :CH])
                            tmp = hfin.tile([128, CH], f32, tag="tmp")
                            nc.vector.tensor_tensor(
                                out=tmp[:nb, :], in0=psums[b][:nb, :],
                                in1=st[:nb, :], op=mybir.AluOpType.add)
                            if dest_f16:
                                ot = hfin.tile([128, TW], f16, tag="ot")
                                nc.vector.memset(ot[:, CH:], 0.0)
                                nc.vector.tensor_scalar(
                                    out=ot[:nb, :CH], in0=tmp[:nb, :],
                                    scalar1=scale_sb[:nb, blk:blk + 1],
                                    scalar2=None,
                                    op0=mybir.AluOpType.mult)
                                nc.sync.dma_start(
                                    out=dest[blk * 128:blk * 128 + nb, :],
                                    in_=ot[:nb, :])
                            else:
                                ot = hfin.tile([128, CH], f32, tag="otf")
                                nc.vector.tensor_scalar(
                                    out=ot[:nb, :], in0=tmp[:nb, :],
                                    scalar1=scale_sb[:nb, blk:blk + 1],
                                    scalar2=None,
                                    op0=mybir.AluOpType.mult)
                                nc.sync.dma_start(
                                    out=dest[blk * 128:blk * 128 + nb, :],
                                    in_=ot[:nb, :])

            hop(u_full, u_self, dinv2_sb, u2_self, True)
            nc.gpsimd.collective_compute(
                "AllGather", mybir.AluOpType.bypass,
                ins=[u2_self[:]], outs=[u2_full[:]], replica_groups=groups)
            hop(u2_full, u2_self, dinv_sb, out_d, False)

    nc.compile()
    return nc


# ---------------------------------------------------------------- entry points

def make_in_maps(edge_index, x, W, p, prep):
    NCORES, RPC = p["NCORES"], p["RPC"]
    x = np.asarray(x, dtype=np.float32)
    W_arr = np.ascontiguousarray(np.asarray(W, dtype=np.float16))
    in_maps = []
    for c in range(NCORES):
        in_maps.append({
            "xT": np.ascontiguousarray(x[c * RPC:(c + 1) * RPC].T.astype(np.float16)),
            "W": W_arr,
            "gidx": prep["gidx_w"][c],
            "rloc": prep["rloc_w"][c],
            "dinv_a": prep["dinv_a"][c],
            "dinv2_a": prep["dinv2_a"][c],
            "iota": prep["iota"],
        })
    return in_maps


_CACHE = {}


def _run(edge_index, x, W, trace=False, tmpdir=None):
    from concourse.bass_utils import run_bass_kernel_spmd

    p = derive_params(full_params())
    prep = preprocess(edge_index, p)
    in_maps = make_in_maps(edge_index, x, W, p, prep)

    key = ("nc", prep["CPB"])
    if key not in _CACHE:
        _CACHE[key] = build_nc(p, prep["CPB"])
    nc = _CACHE[key]

    res = run_bass_kernel_spmd(nc, in_maps, list(range(p["NCORES"])),
                               trace=trace, tmpdir=tmpdir)
    out = np.concatenate([res.results[c]["out"] for c in range(p["NCORES"])], axis=0)
    return out, res


def kernel(edge_index, x, W):
    out, _ = _run(edge_index, x, W)
    return out


# revision 8
# speedup vs baseline: 2.2018x; 2.2018x over previous
"""Trainium2 Bass kernel for a 2-hop GCN-normalized aggregation (Nettack surrogate).

Computes h = A_hat^2 (x @ W) with A_hat = D^-1/2 (A + I) D^-1/2, where A is the
(self-loop-stripped) adjacency from edge_index, on 8 NeuronCores.

Strategy:
  - Nodes are partitioned row-wise across 8 cores (12500 rows each).
  - u = dinv * (x @ W) computed per-core (x slice shipped pre-transposed, fp16),
    stored as an fp16 table padded to 128 channels (256B rows), AllGather ->
    full u table per core.
  - Each hop: per destination-node block of 128, gather u[col] rows for that
    block's edges via SWDGE dma_gather (int16 indices => 4 source sub-ranges,
    spread over the 4 SWDGE queue pairs), build a 0/1 one-hot (edge -> local
    row) with DVE is_equal in fp16, segment-sum via fp16 PE matmul accumulating
    in fp32 PSUM. Add self term, scale by dinv^2 (hop1) / dinv (hop2).
  - AllGather between hops; final f32 result is the output slice per core.

Host-side preprocessing is limited to edge metadata: bucketing/padding edge
indices, degree counts, and layout packing. All FLOPs on x/W run on device.
"""

import numpy as np


# ---------------------------------------------------------------- parameters

def full_params():
    return dict(
        N=100000, E=3200000, IN_CH=512, CH=64, NCORES=8,
        SBB=2,    # 128-row blocks per superblock (gather granularity)
        NR=4,     # int16 index sub-ranges
    )


def derive_params(p):
    q = dict(p)
    q["RPC"] = p["N"] // p["NCORES"]                      # rows per core
    q["NBLK"] = -(-q["RPC"] // 128)                       # 128-row blocks per core
    assert q["NBLK"] % p["SBB"] == 0, (q["NBLK"], p["SBB"])
    q["NSB"] = q["NBLK"] // p["SBB"]
    assert p["N"] % p["NR"] == 0
    q["RANGE"] = p["N"] // p["NR"]
    assert q["RANGE"] <= 32768
    q["KT"] = p["IN_CH"] // 128
    assert p["IN_CH"] % 128 == 0
    q["TW"] = 128                                         # fp16 table row width (256B)
    assert p["CH"] <= q["TW"]
    return q


# ---------------------------------------------------------------- host prep

def preprocess(edge_index, p):
    """Edge metadata -> per-core packed index/one-hot-row/scale arrays."""
    N, NCORES, NR = p["N"], p["NCORES"], p["NR"]
    RPC, NBLK, SBB, NSB, RANGE = p["RPC"], p["NBLK"], p["SBB"], p["NSB"], p["RANGE"]

    row = np.asarray(edge_index[0], dtype=np.int64)
    col = np.asarray(edge_index[1], dtype=np.int64)
    keep = row != col
    row, col = row[keep], col[keep]

    deg = np.bincount(row, minlength=N).astype(np.float64) + 1.0
    dinv = (1.0 / np.sqrt(deg)).astype(np.float32)

    core = row // RPC
    lrow = row - core * RPC
    blk = lrow >> 7
    rloc = (lrow & 127).astype(np.float32)
    rng_ = col // RANGE

    bucket = (core * NBLK + blk) * NR + rng_
    counts = np.bincount(bucket, minlength=NCORES * NBLK * NR)
    CPB = max(1, int(-(-counts.max() // 128)))
    CAP = CPB * 128

    order = np.argsort(bucket, kind="stable")
    sortb = bucket[order]
    starts = np.zeros(len(counts) + 1, np.int64)
    np.cumsum(counts, out=starts[1:])
    within = np.arange(len(row), dtype=np.int64) - starts[sortb]

    c_s = sortb // (NBLK * NR)
    rem = sortb % (NBLK * NR)
    blk_s = rem // NR
    r_s = rem % NR
    sb_s = blk_s // SBB
    bin_s = blk_s % SBB
    F = ((sb_s * NR + r_s) * SBB + bin_s) * CAP + within

    L = NSB * NR * SBB * CAP
    gidx = np.zeros((NCORES, L), np.int16)            # pad: valid dummy row 0
    rl = np.full((NCORES, L), -1.0, np.float16)       # pad: matches no row
    gidx[c_s, F] = (col[order] - r_s * RANGE).astype(np.int16)
    rl[c_s, F] = rloc[order]

    # wrap: idx i -> [i%16, i//16], replicated into all 8 groups of 16 rows
    gidx_w = np.zeros((NCORES, 128, L // 16), np.int16)
    base = gidx.reshape(NCORES, L // 16, 16).transpose(0, 2, 1)
    for g in range(8):
        gidx_w[:, g * 16:(g + 1) * 16, :] = base
    rloc_w = np.ascontiguousarray(rl.reshape(NCORES, L // 128, 128).transpose(0, 2, 1))

    # per-core dinv / dinv^2 in [128, NBLK] block layout (pad rows -> 0)
    dpad = np.zeros(NCORES * NBLK * 128, np.float32)
    dpad_view = dpad.reshape(NCORES, NBLK * 128)
    for c in range(NCORES):
        dpad_view[c, :RPC] = dinv[c * RPC:(c + 1) * RPC]
    dinv_a = np.ascontiguousarray(dpad.reshape(NCORES, NBLK, 128).transpose(0, 2, 1))
    dinv2_a = dinv_a * dinv_a

    iota = np.broadcast_to(np.arange(128, dtype=np.float16), (128, 128)).copy()
    return dict(CPB=CPB, gidx_w=gidx_w, rloc_w=rloc_w,
                dinv_a=dinv_a, dinv2_a=dinv2_a, iota=iota)


# ---------------------------------------------------------------- device program

def build_nc(p, CPB):
    import concourse.bacc as bacc
    import concourse.mybir as mybir
    from concourse.tile import TileContext

    f32 = mybir.dt.float32
    f16 = mybir.dt.float16
    N, CH, IN_CH, NCORES = p["N"], p["CH"], p["IN_CH"], p["NCORES"]
    RPC, NBLK, SBB, NSB, NR, RANGE, KT, TW = (
        p["RPC"], p["NBLK"], p["SBB"], p["NSB"], p["NR"], p["RANGE"], p["KT"],
        p["TW"])
    CAP = CPB * 128
    SECT = SBB * CPB            # chunk columns per (sb, range) section
    L = NSB * NR * SBB * CAP    # per-core edge-slot count

    nc = bacc.Bacc("TRN2", num_swdge_queues=4)
    xT_d = nc.declare_dram_parameter("xT", [IN_CH, RPC], f16, isOutput=False)
    W_d = nc.declare_dram_parameter("W", [IN_CH, CH], f16, isOutput=False)
    gidx_d = nc.declare_dram_parameter("gidx", [128, L // 16], mybir.dt.int16, isOutput=False)
    rloc_d = nc.declare_dram_parameter("rloc", [128, L // 128], f16, isOutput=False)
    dinv_d = nc.declare_dram_parameter("dinv_a", [128, NBLK], f32, isOutput=False)
    dinv2_d = nc.declare_dram_parameter("dinv2_a", [128, NBLK], f32, isOutput=False)
    iota_d = nc.declare_dram_parameter("iota", [128, 128], f16, isOutput=False)
    out_d = nc.declare_dram_parameter("out", [RPC, CH], f32, isOutput=True)

    u_self = nc.dram_tensor("u_self", [RPC, TW], f16)
    u2_self = nc.dram_tensor("u2_self", [RPC, TW], f16)
    u_full = nc.dram_tensor("u_full", [N, TW], f16, addr_space="Shared")
    u2_full = nc.dram_tensor("u2_full", [N, TW], f16, addr_space="Shared")

    groups = [list(range(NCORES))]

    with TileContext(nc) as tc:
        with tc.tile_pool(name="const", bufs=1) as cpool:
            iota_sb = cpool.tile([128, 128], f16)
            nc.sync.dma_start(out=iota_sb[:], in_=iota_d[:])
            dinv_sb = cpool.tile([128, NBLK], f32)
            nc.sync.dma_start(out=dinv_sb[:], in_=dinv_d[:])
            dinv2_sb = cpool.tile([128, NBLK], f32)
            nc.sync.dma_start(out=dinv2_sb[:], in_=dinv2_d[:])
            w_sb = cpool.tile([128, KT, CH], f16)
            for k in range(KT):
                nc.sync.dma_start(out=w_sb[:, k, :], in_=W_d[k * 128:(k + 1) * 128, :])

            # ---------------- phase 1: u = dinv * (x @ W) ----------------
            with tc.tile_pool(name="p1", bufs=4) as p1, \
                 tc.tile_pool(name="p1ps", bufs=2, space="PSUM") as p1ps, \
                 tc.tile_pool(name="p1o", bufs=3) as p1o:
                for blk in range(NBLK):
                    nb = min(128, RPC - blk * 128)
                    ps = p1ps.tile([128, CH], f32, tag="ps")
                    for k in range(KT):
                        xt = p1.tile([128, 128], f16, tag="xt")
                        nc.sync.dma_start(
                            out=xt[:, :nb],
                            in_=xT_d[k * 128:(k + 1) * 128, blk * 128:blk * 128 + nb])
                        nc.tensor.matmul(
                            out=ps[:nb, :], lhsT=xt[:, :nb], rhs=w_sb[:, k, :],
                            start=(k == 0), stop=(k == KT - 1))
                    ut = p1o.tile([128, TW], f16, tag="ut")
                    nc.vector.memset(ut[:, CH:], 0.0)
                    nc.vector.tensor_scalar(
                        out=ut[:nb, :CH], in0=ps[:nb, :],
                        scalar1=dinv_sb[:nb, blk:blk + 1], scalar2=None,
                        op0=mybir.AluOpType.mult)
                    nc.sync.dma_start(out=u_self[blk * 128:blk * 128 + nb, :],
                                      in_=ut[:nb, :])

            nc.gpsimd.collective_compute(
                "AllGather", mybir.AluOpType.bypass,
                ins=[u_self[:]], outs=[u_full[:]], replica_groups=groups)

            # ---------------- hops ----------------
            def hop(table, self_tab, scale_sb, dest, dest_f16):
                with tc.tile_pool(name="hidx", bufs=3) as hidx, \
                     tc.tile_pool(name="hrl", bufs=3) as hrl, \
                     tc.tile_pool(name="hg", bufs=3) as hg, \
                     tc.tile_pool(name="hoh", bufs=4) as hoh, \
                     tc.tile_pool(name="hps", bufs=1, space="PSUM") as hps, \
                     tc.tile_pool(name="hfin", bufs=3) as hfin:
                    for sb in range(NSB):
                        cpsb = NR * SECT          # chunk cols per sb
                        idx_sb = hidx.tile([128, cpsb * 8], mybir.dt.int16, tag="idx")
                        nc.sync.dma_start(
                            out=idx_sb[:],
                            in_=gidx_d[:, sb * cpsb * 8:(sb + 1) * cpsb * 8])
                        rl_sb = hrl.tile([128, cpsb], f16, tag="rl")
                        nc.sync.dma_start(
                            out=rl_sb[:],
                            in_=rloc_d[:, sb * cpsb:(sb + 1) * cpsb])
                        psums = [hps.tile([128, CH], f32, tag=f"hp{b}",
                                          name=f"hp{b}")
                                 for b in range(SBB)]
                        for r in range(NR):
                            g = hg.tile([128, SECT, TW], f16, tag=f"g{r}",
                                        name=f"g{r}")
                            nc.gpsimd.dma_gather(
                                g[:], table[r * RANGE:(r + 1) * RANGE, :],
                                idx_sb[:, r * SECT * 8:(r + 1) * SECT * 8],
                                num_idxs=SECT * 128, num_idxs_reg=SECT * 128,
                                elem_size=TW, single_packet=False,
                                queue_num=r)
                            oh = hoh.tile([128, SECT, 128], f16, tag="oh")
                            nc.vector.tensor_tensor(
                                out=oh[:],
                                in0=iota_sb[:].unsqueeze(1)
                                    .broadcast_to([128, SECT, 128]),
                                in1=rl_sb[:, r * SECT:(r + 1) * SECT]
                                    .unsqueeze(-1).broadcast_to([128, SECT, 128]),
                                op=mybir.AluOpType.is_equal)
                            for b in range(SBB):
                                for j in range(CPB):
                                    ch = b * CPB + j
                                    nc.tensor.matmul(
                                        out=psums[b][:, :],
                                        lhsT=oh[:, ch, :],
                                        rhs=g[:, ch, :CH],
                                        start=(r == 0 and j == 0),
                                        stop=(r == NR - 1 and j == CPB - 1))
                        for b in range(SBB):
                            blk = sb * SBB + b
                            nb = min(128, RPC - blk * 128)
                            if nb <= 0:
                                continue
                            st = hfin.tile([128, CH], f16, tag="st")
                            nc.sync.dma_start(
                                out=st[:nb, :],
                                in_=self_tab[blk * 128:blk * 128 + nb, :CH])
                            tmp = hfin.tile([128, CH], f32, tag="tmp")
                            nc.vector.tensor_tensor(
                                out=tmp[:nb, :], in0=psums[b][:nb, :],
                                in1=st[:nb, :], op=mybir.AluOpType.add)
                            if dest_f16:
                                ot = hfin.tile([128, TW], f16, tag="ot")
                                nc.vector.memset(ot[:, CH:], 0.0)
                                nc.vector.tensor_scalar(
                                    out=ot[:nb, :CH], in0=tmp[:nb, :],
                                    scalar1=scale_sb[:nb, blk:blk + 1],
                                    scalar2=None,
                                    op0=mybir.AluOpType.mult)
                                nc.sync.dma_start(
                                    out=dest[blk * 128:blk * 128 + nb, :],
                                    in_=ot[:nb, :])
                            else:
                                ot = hfin.tile([128, CH], f32, tag="otf")
                                nc.vector.tensor_scalar(
                                    out=ot[:nb, :], in0=tmp[:nb, :],
                                    scalar1=scale_sb[:nb, blk:blk + 1],
                                    scalar2=None,
                                    op0=mybir.AluOpType.mult)
                                nc.sync.dma_start(
                                    out=dest[blk * 128:blk * 128 + nb, :],
                                    in_=ot[:nb, :])

            hop(u_full, u_self, dinv2_sb, u2_self, True)
            nc.gpsimd.collective_compute(
                "AllGather", mybir.AluOpType.bypass,
                ins=[u2_self[:]], outs=[u2_full[:]], replica_groups=groups)
            hop(u2_full, u2_self, dinv_sb, out_d, False)

    nc.compile()
    return nc


# ---------------------------------------------------------------- entry points

def make_in_maps(edge_index, x, W, p, prep):
    NCORES, RPC = p["NCORES"], p["RPC"]
    x = np.asarray(x, dtype=np.float32)
    W_arr = np.ascontiguousarray(np.asarray(W, dtype=np.float16))
    in_maps = []
    for c in range(NCORES):
        in_maps.append({
            "xT": np.ascontiguousarray(x[c * RPC:(c + 1) * RPC].T.astype(np.float16)),
            "W": W_arr,
            "gidx": prep["gidx_w"][c],
            "rloc": prep["rloc_w"][c],
            "dinv_a": prep["dinv_a"][c],
            "dinv2_a": prep["dinv2_a"][c],
            "iota": prep["iota"],
        })
    return in_maps


_CACHE = {}


def _run(edge_index, x, W, trace=False, tmpdir=None):
    from concourse.bass_utils import run_bass_kernel_spmd

    p = derive_params(full_params())
    prep = preprocess(edge_index, p)
    in_maps = make_in_maps(edge_index, x, W, p, prep)

    key = ("nc", prep["CPB"])
    if key not in _CACHE:
        _CACHE[key] = build_nc(p, prep["CPB"])
    nc = _CACHE[key]

    res = run_bass_kernel_spmd(nc, in_maps, list(range(p["NCORES"])),
                               trace=trace, tmpdir=tmpdir)
    out = np.concatenate([res.results[c]["out"] for c in range(p["NCORES"])], axis=0)
    return out, res


def kernel(edge_index, x, W):
    out, _ = _run(edge_index, x, W)
    return out
